# revision 1
# baseline (speedup 1.0000x reference)
"""Trainium2 Bass kernel for nn_Hierarch_RNN (hierarchical 2-layer GRU).

Strategy: data-parallel over the batch dim (32 batches -> 4 per core, 8 cores).
On-chip layout is feature-major [d, rows]; rows per core = 4*321 = 1284,
processed in 3 chunks of 428 columns (PSUM bank = 512 fp32 max).

Per GRU step (both layers, encoder + decoder reuse the same emitter):
  - x-side and h-side matmuls accumulate into one PSUM group for the r/z
    gates (sigmoid applied straight from PSUM with the bias via ScalarE).
  - n gate keeps x/h sides in separate PSUM tiles; fused DVE
    scalar_tensor_tensor computes (h_n + bhh_n) * r; tanh adds bih_n.
  - h' = n + z*(h - n) in three DVE tensor-tensor ops per block-chunk.
All matmuls run in float32r (full-rate fp32 mode, ~1e-4 rel err).
"""
import numpy as np

import concourse.mybir as mybir
import concourse.tile as tile
from concourse import bacc
from concourse.bass_utils import run_bass_kernel_spmd

F32 = mybir.dt.float32
F32R = mybir.dt.float32r
AF = mybir.ActivationFunctionType
ALU = mybir.AluOpType

B, SEQ, PRED, ENC = 32, 720, 96, 321
NCORE, BPC = 8, 4
R = BPC * ENC                 # 1284 rows per core
CH, NCH = 428, 3              # row chunks
# layer params: d, seg_len, n 128-blocks of d (DK == NG), decoder steps S
D0, SG0, DK0, S0, T0 = 512, 48, 4, 2, 15
D1, SG1, DK1, S1, T1 = 256, 24, 2, 4, 60

_CACHE = {}


def _build_nc(l0_steps=T0, l1_steps=T1):
    nc = bacc.Bacc("TRN2", target_bir_lowering=False, debug=False,
                   num_devices=NCORE)

    # ---------------- DRAM tensors ----------------
    xseg0_d = nc.dram_tensor("xseg0", [T0, SG0, R], F32R, kind="ExternalInput")
    xseg1_d = nc.dram_tensor("xseg1", [SG1, 4 * R], F32R, kind="ExternalInput")
    wih_d = [nc.dram_tensor("wihT0", [DK0, 128, 3 * D0], F32R, kind="ExternalInput"),
             nc.dram_tensor("wihT1", [DK1, 128, 3 * D1], F32R, kind="ExternalInput")]
    whh_d = [nc.dram_tensor("whhT0", [DK0, 128, 3 * D0], F32R, kind="ExternalInput"),
             nc.dram_tensor("whhT1", [DK1, 128, 3 * D1], F32R, kind="ExternalInput")]
    wemb_d = [nc.dram_tensor("wembT0", [SG0, D0], F32R, kind="ExternalInput"),
              nc.dram_tensor("wembT1", [SG1, D1], F32R, kind="ExternalInput")]
    wpred_d = [nc.dram_tensor("wpredT0", [DK0, 128, SG0], F32R, kind="ExternalInput"),
               nc.dram_tensor("wpredT1", [DK1, 128, SG1], F32R, kind="ExternalInput")]
    brz_d = [nc.dram_tensor("brz0", [128, 2 * DK0], F32, kind="ExternalInput"),
             nc.dram_tensor("brz1", [128, 2 * DK1], F32, kind="ExternalInput")]
    bihn_d = [nc.dram_tensor("bihn0", [128, DK0], F32, kind="ExternalInput"),
              nc.dram_tensor("bihn1", [128, DK1], F32, kind="ExternalInput")]
    bhhn_d = [nc.dram_tensor("bhhn0", [128, DK0], F32, kind="ExternalInput"),
              nc.dram_tensor("bhhn1", [128, DK1], F32, kind="ExternalInput")]
    bemb_d = [nc.dram_tensor("bemb0", [128, DK0], F32, kind="ExternalInput"),
              nc.dram_tensor("bemb1", [128, DK1], F32, kind="ExternalInput")]
    bpred_d = [nc.dram_tensor("bpred0", [128, 1], F32, kind="ExternalInput"),
               nc.dram_tensor("bpred1", [128, 1], F32, kind="ExternalInput")]
    posx_d = [nc.dram_tensor("posx0", [S0, DK0, 128, R], F32R, kind="ExternalInput"),
              nc.dram_tensor("posx1", [S1, DK1, 128, R], F32R, kind="ExternalInput")]
    y_d = [nc.dram_tensor("y0", [S0, SG0, R], F32, kind="ExternalOutput"),
           nc.dram_tensor("y1", [S1, SG1, R], F32, kind="ExternalOutput")]

    with tile.TileContext(nc) as tc:
        with tc.tile_pool(name="const", bufs=1) as cp, \
             tc.tile_pool(name="x0p", bufs=1) as x0p, \
             tc.tile_pool(name="xep", bufs=6) as xep, \
             tc.tile_pool(name="h0p", bufs=8) as h0p, \
             tc.tile_pool(name="h1p", bufs=4) as h1p, \
             tc.tile_pool(name="posp", bufs=5) as posp, \
             tc.tile_pool(name="hyp", bufs=6) as hyp, \
             tc.tile_pool(name="rp", bufs=2) as rp, \
             tc.tile_pool(name="zp", bufs=2) as zp, \
             tc.tile_pool(name="np_", bufs=2) as np_p, \
             tc.tile_pool(name="sp", bufs=2) as sp, \
             tc.tile_pool(name="tp", bufs=2) as tp, \
             tc.tile_pool(name="up", bufs=2) as up, \
             tc.tile_pool(name="vp", bufs=2) as vp, \
             tc.tile_pool(name="yp", bufs=2) as yp, \
             tc.tile_pool(name="psg", bufs=6, space="PSUM") as psg, \
             tc.tile_pool(name="psy", bufs=2, space="PSUM") as psy:

            # ---------------- load constants ----------------
            def load_w(dram, k_tiles, cols):
                t = cp.tile([128, k_tiles * cols], F32R, tag=f"c_{dram.name}",
                            name=f"c_{dram.name}")
                for k in range(k_tiles):
                    nc.sync.dma_start(t[:, k * cols:(k + 1) * cols], dram[k])
                return t

            wih_sb = [load_w(wih_d[0], DK0, 3 * D0), load_w(wih_d[1], DK1, 3 * D1)]
            whh_sb = [load_w(whh_d[0], DK0, 3 * D0), load_w(whh_d[1], DK1, 3 * D1)]
            wpred_sb = [load_w(wpred_d[0], DK0, SG0), load_w(wpred_d[1], DK1, SG1)]
            wemb_sb = []
            for li, (sg, d) in enumerate(((SG0, D0), (SG1, D1))):
                t = cp.tile([sg, d], F32R, tag=f"c_wemb{li}", name=f"c_wemb{li}")
                nc.sync.dma_start(t[:], wemb_d[li][:])
                wemb_sb.append(t)
            def load_b(dram, cols):
                t = cp.tile([128, cols], F32, tag=f"c_{dram.name}",
                            name=f"c_{dram.name}")
                nc.sync.dma_start(t[:], dram[:])
                return t
            brz_sb = [load_b(brz_d[0], 2 * DK0), load_b(brz_d[1], 2 * DK1)]
            bihn_sb = [load_b(bihn_d[0], DK0), load_b(bihn_d[1], DK1)]
            bhhn_sb = [load_b(bhhn_d[0], DK0), load_b(bhhn_d[1], DK1)]
            bemb_sb = [load_b(bemb_d[0], DK0), load_b(bemb_d[1], DK1)]
            bpred_sb = [load_b(bpred_d[0], 1), load_b(bpred_d[1], 1)]
            xs1 = cp.tile([SG1, 4 * R], F32R, tag="c_xs1", name="c_xs1")
            nc.sync.dma_start(xs1[:], xseg1_d[:])

            LP = [dict(D=D0, DK=DK0, SG=SG0, wih=wih_sb[0], whh=whh_sb[0],
                       wemb=wemb_sb[0], wpred=wpred_sb[0], brz=brz_sb[0],
                       bihn=bihn_sb[0], bhhn=bhhn_sb[0], bemb=bemb_sb[0],
                       bpred=bpred_sb[0]),
                  dict(D=D1, DK=DK1, SG=SG1, wih=wih_sb[1], whh=whh_sb[1],
                       wemb=wemb_sb[1], wpred=wpred_sb[1], brz=brz_sb[1],
                       bihn=bihn_sb[1], bhhn=bhhn_sb[1], bemb=bemb_sb[1],
                       bpred=bpred_sb[1])]

            def wcol(P, wt, k, m):
                """AP of [128,128] weight block: k-tile k, m-tile m of 3d."""
                c0 = k * 3 * P["D"] + m * 128
                return wt[:, c0:c0 + 128]

            def make_xe_embed(li, xsrc_fn):
                """Returns make_xe(c): emits per-chunk embed, returns DK APs."""
                P = LP[li]
                def make_xe(c):
                    aps = []
                    for k in range(P["DK"]):
                        ps = psg.tile([128, CH], F32, tag="ps", name="ps_e")
                        nc.tensor.matmul(ps[:], P["wemb"][:, k * 128:(k + 1) * 128],
                                         xsrc_fn(c), start=True, stop=True)
                        xe = xep.tile([128, CH], F32R, tag="xe", name=f"xe{li}_{k}")
                        nc.scalar.activation(xe[:], ps[:], AF.Relu,
                                             bias=P["bemb"][:, k:k + 1])
                        aps.append(xe[:])
                    return aps
                return make_xe

            def emit_gru(li, make_xe, h_in, hout_ap, first):
                """One fused GRU application over all chunks/blocks.
                make_xe(c) -> list of DK x-side rhs APs [128, CH]
                h_in: list of DK [128, R] tiles (prev h), or None if first
                hout_ap(i, c): output AP [128, CH] (f32r tile slice)
                """
                P = LP[li]
                DK = P["DK"]
                for c in range(NCH):
                    cc = slice(c * CH, (c + 1) * CH)
                    xe = make_xe(c)
                    for i in range(DK):
                        # --- r gate (m = i) ---
                        ps_r = psg.tile([128, CH], F32, tag="ps", name="ps_r")
                        for k in range(DK):
                            nc.tensor.matmul(ps_r[:], wcol(P, P["wih"], k, i),
                                             xe[k], start=(k == 0),
                                             stop=(k == DK - 1 and first))
                        if not first:
                            for k in range(DK):
                                nc.tensor.matmul(ps_r[:], wcol(P, P["whh"], k, i),
                                                 h_in[k][:, cc], start=False,
                                                 stop=(k == DK - 1))
                        r = rp.tile([128, CH], F32)
                        nc.scalar.activation(r[:], ps_r[:], AF.Sigmoid,
                                             bias=P["brz"][:, i:i + 1])
                        # --- z gate (m = DK + i) ---
                        ps_z = psg.tile([128, CH], F32, tag="ps", name="ps_z")
                        for k in range(DK):
                            nc.tensor.matmul(ps_z[:], wcol(P, P["wih"], k, DK + i),
                                             xe[k], start=(k == 0),
                                             stop=(k == DK - 1 and first))
                        if not first:
                            for k in range(DK):
                                nc.tensor.matmul(ps_z[:], wcol(P, P["whh"], k, DK + i),
                                                 h_in[k][:, cc], start=False,
                                                 stop=(k == DK - 1))
                        z = zp.tile([128, CH], F32)
                        nc.scalar.activation(z[:], ps_z[:], AF.Sigmoid,
                                             bias=P["brz"][:, DK + i:DK + i + 1])
                        # --- n gate (m = 2*DK + i) ---
                        ps_in = psg.tile([128, CH], F32, tag="ps", name="ps_in")
                        for k in range(DK):
                            nc.tensor.matmul(ps_in[:], wcol(P, P["wih"], k, 2 * DK + i),
                                             xe[k], start=(k == 0),
                                             stop=(k == DK - 1))
                        t_ = tp.tile([128, CH], F32)
                        if first:
                            nc.vector.tensor_scalar_mul(t_[:], r[:],
                                                        P["bhhn"][:, i:i + 1])
                        else:
                            ps_hn = psg.tile([128, CH], F32, tag="ps", name="ps_hn")
                            for k in range(DK):
                                nc.tensor.matmul(ps_hn[:], wcol(P, P["whh"], k, 2 * DK + i),
                                                 h_in[k][:, cc], start=(k == 0),
                                                 stop=(k == DK - 1))
                            nc.vector.scalar_tensor_tensor(
                                t_[:], ps_hn[:], P["bhhn"][:, i:i + 1], r[:],
                                op0=ALU.add, op1=ALU.mult)
                        s_ = sp.tile([128, CH], F32)
                        nc.vector.tensor_add(s_[:], t_[:], ps_in[:])
                        n = np_p.tile([128, CH], F32)
                        nc.scalar.activation(n[:], s_[:], AF.Tanh,
                                             bias=P["bihn"][:, i:i + 1])
                        # --- h' = n + z*(h-n)  (h=0 when first) ---
                        if first:
                            v = vp.tile([128, CH], F32)
                            nc.vector.tensor_mul(v[:], n[:], z[:])
                            nc.vector.tensor_sub(hout_ap(i, c), n[:], v[:])
                        else:
                            u = up.tile([128, CH], F32)
                            nc.vector.tensor_sub(u[:], h_in[i][:, cc], n[:])
                            v = vp.tile([128, CH], F32)
                            nc.vector.tensor_mul(v[:], u[:], z[:])
                            nc.vector.tensor_add(hout_ap(i, c), n[:], v[:])

            def emit_enc_step(li, t, make_xe, h_in):
                P = LP[li]
                h_pool = h0p if li == 0 else h1p
                h_out = [h_pool.tile([128, R], F32R, tag=f"h{li}", name=f"h{li}_{t}_{k}")
                         for k in range(P["DK"])]
                emit_gru(li, make_xe,
                         h_in, lambda i, c: h_out[i][:, c * CH:(c + 1) * CH],
                         first=(t == 0))
                return h_out

            def emit_decoder(li, s_, h_fin):
                P = LP[li]
                DK, SG = P["DK"], P["SG"]
                hy = {}
                def hout(i, c):
                    t = hyp.tile([128, CH], F32R, tag="hy", name=f"hy{li}_{s_}_{i}_{c}")
                    hy[(i, c)] = t
                    return t[:]
                def make_xe(c):
                    aps = []
                    for k in range(DK):
                        pt = posp.tile([128, CH], F32R, tag="pos", name=f"pos{li}_{s_}_{k}_{c}")
                        nc.sync.dma_start(pt[:], posx_d[li][s_, k, :, c * CH:(c + 1) * CH])
                        aps.append(pt[:])
                    return aps
                emit_gru(li, make_xe, h_fin, hout, first=False)
                for c in range(NCH):
                    cc = slice(c * CH, (c + 1) * CH)
                    ps = psy.tile([SG, CH], F32, tag="psy", name="ps_y")
                    for k in range(DK):
                        nc.tensor.matmul(ps[:], P["wpred"][:, k * SG:(k + 1) * SG],
                                         hy[(k, c)][:], start=(k == 0),
                                         stop=(k == DK - 1))
                    y = yp.tile([SG, CH], F32)
                    nc.scalar.activation(y[:], ps[:], AF.Identity,
                                         bias=P["bpred"][0:SG, 0:1])
                    nc.sync.dma_start(y_d[li][s_, :, cc], y[:])

            # ---------------- encoder ----------------
            h0 = None
            h1 = None
            t1 = 0
            for t in range(l0_steps):
                xs_t = x0p.tile([SG0, R], F32R)
                nc.sync.dma_start(xs_t[:], xseg0_d[t])
                h0 = emit_enc_step(
                    0, t, make_xe_embed(0, lambda c, xs_t=xs_t: xs_t[:, c * CH:(c + 1) * CH]),
                    h0)
                for _ in range(4):
                    if t1 < l1_steps:
                        j = t1 % 4
                        h1 = emit_enc_step(
                            1, t1,
                            make_xe_embed(1, lambda c, j=j: xs1[:, j * R + c * CH:j * R + (c + 1) * CH]),
                            h1)
                        t1 += 1
            while t1 < l1_steps:
                j = t1 % 4
                h1 = emit_enc_step(
                    1, t1,
                    make_xe_embed(1, lambda c, j=j: xs1[:, j * R + c * CH:j * R + (c + 1) * CH]),
                    h1)
                t1 += 1

            # ---------------- decoders ----------------
            emit_decoder(0, 0, h0)
            emit_decoder(1, 0, h1)
            emit_decoder(0, 1, h0)
            emit_decoder(1, 1, h1)
            emit_decoder(1, 2, h1)
            emit_decoder(1, 3, h1)

    nc.compile()
    return nc


def get_nc(l0_steps=T0, l1_steps=T1):
    key = (l0_steps, l1_steps)
    if key not in _CACHE:
        _CACHE[key] = _build_nc(l0_steps, l1_steps)
    return _CACHE[key]


# ==================== host side ====================

def _prep_shared(inp):
    f = np.float32
    m = {}
    for li, d in ((0, D0), (1, D1)):
        dk = (DK0, DK1)[li]
        sg = (SG0, SG1)[li]
        m[f"wembT{li}"] = np.ascontiguousarray(inp[f"W_emb{li}"].T, f)
        m[f"wihT{li}"] = np.ascontiguousarray(
            inp[f"Wih{li}"].T.reshape(dk, 128, 3 * d), f)
        m[f"whhT{li}"] = np.ascontiguousarray(
            inp[f"Whh{li}"].T.reshape(dk, 128, 3 * d), f)
        m[f"wpredT{li}"] = np.ascontiguousarray(
            inp[f"Wpred{li}"].T.reshape(dk, 128, sg), f)
        bih, bhh = inp[f"bih{li}"].astype(f), inp[f"bhh{li}"].astype(f)
        m[f"brz{li}"] = np.ascontiguousarray(
            (bih + bhh)[:2 * d].reshape(2 * dk, 128).T)
        m[f"bihn{li}"] = np.ascontiguousarray(bih[2 * d:].reshape(dk, 128).T)
        m[f"bhhn{li}"] = np.ascontiguousarray(bhh[2 * d:].reshape(dk, 128).T)
        m[f"bemb{li}"] = np.ascontiguousarray(
            inp[f"b_emb{li}"].astype(f).reshape(dk, 128).T)
        bp = np.zeros((128, 1), f)
        bp[:sg, 0] = inp[f"bpred{li}"].astype(f)
        m[f"bpred{li}"] = bp
        half = d // 2
        pos, chan = inp[f"pos{li}"].astype(f), inp[f"chan{li}"].astype(f)
        S = pos.shape[0]
        base = np.concatenate(
            [np.broadcast_to(pos[:, None, :], (S, ENC, half)),
             np.broadcast_to(chan[None, :, :], (S, ENC, half))], axis=-1)
        posx = np.tile(base.transpose(0, 2, 1), (1, 1, BPC))   # [S, d, R]
        m[f"posx{li}"] = np.ascontiguousarray(posx.reshape(S, dk, 128, R))
    return m


def _prep_core(x, c):
    f = np.float32
    xb = x[BPC * c:BPC * (c + 1)].astype(f)
    last = xb[:, -1:, :]
    xc = (xb - last).transpose(0, 2, 1).reshape(R, SEQ)
    xseg0 = np.ascontiguousarray(xc.reshape(R, T0, SG0).transpose(1, 2, 0))
    xseg1 = np.ascontiguousarray(
        xc[:, :4 * SG1].reshape(R, 4, SG1).transpose(2, 1, 0).reshape(SG1, 4 * R))
    return xseg0, xseg1


def kernel(**inputs):
    x = np.asarray(inputs["x"], np.float32)
    shared = _prep_shared({k: np.asarray(v) for k, v in inputs.items()})
    in_maps = []
    for c in range(NCORE):
        xseg0, xseg1 = _prep_core(x, c)
        in_maps.append({"xseg0": xseg0, "xseg1": xseg1, **shared})
    nc = get_nc()
    res = run_bass_kernel_spmd(nc, in_maps, list(range(NCORE))).results
    full0 = np.concatenate([res[c]["y0"] for c in range(NCORE)], axis=2)
    full1 = np.concatenate([res[c]["y1"] for c in range(NCORE)], axis=2)
    # out[b, s_*seg+j, e] = y[s_, j, n=(b,e)]
    yl0 = full0.reshape(S0, SG0, B, ENC).transpose(2, 0, 1, 3).reshape(B, PRED, ENC)
    yl1 = full1.reshape(S1, SG1, B, ENC).transpose(2, 0, 1, 3).reshape(B, PRED, ENC)
    return ((yl0 + yl1) / 2.0 + x[:, -1:, :]).astype(np.float32)

